# revision 15
# baseline (speedup 1.0000x reference)
"""ButterflyMlp Trainium2 kernel.

Reference computation (B=65536):
    h1 = relu(x @ (W1*m1).T + b1)          # [B, 784]
    h2 = relu(h1 @ (W2*m2).T + b2)         # [B, 128]
    logits = h2 @ (W3*m3).T + b3           # [B, 10]
    out = log_softmax(logits, axis=1)

Strategy: pure data parallel over 8 NeuronCores (batch sharded 8192/core,
masked weights replicated), activations kept in transposed [features,
batch] layout on-chip.

Key trick vs a dense L1: the butterfly mask for the square 784x784 layer
is nonzero only where (i - o) mod 156 is in {0,1,2} (stripes) or
|i - o| <= 10 (band).  Sorting BOTH feature axes by residue mod 156
maps every nonzero into a circular band of +-55 positions around the
diagonal.  The input features are laid out as 8 ext tiles of
[1 ones-row + 127 payload rows] (payload = the 784 permuted features
edge-replicated by 56 on each side); each 128-wide output tile of W1
then contracts exactly ext tiles (o, o+1), so layer 1 is 7 uniform
DoubleRow fp8 matmuls (K=256) per 512-column batch sub-block instead
of 21 DR + packed-tail passes of a dense kernel.  b1 rides on the
ones-rows, so the PSUM->SBUF evacuations are pure relu and can be
merged across o-tiles: the seven L1 psums live in two [128,2,512]
pairs + one [128,3,512] triple, giving 3 wide relu evacuations + 1 h2
evacuation per sub-block, split across the Scalar and Vector engines.

Layer 2 contracts the 896 permuted h1 rows as 3 DR chunks + 1 plain
K=128 tail (rows 784..895 of W2 are zero, so the h1 pad content is
irrelevant).  The Scalar engine's activation tables are pinned to
natural_log_exp_and_others (relu+exp+ln in one table) so only a single
ACT_TABLE_LOAD ever runs.  log_softmax skips the max-subtraction
(logits are O(5), far from fp32 exp overflow).

The masked weights are pre-scaled by 32 (h1 stored at scale 32, h2 at
1024) to keep fp8 values in the normal range; the scales fold back into
the relu / softmax stages.  The batch is permuted inside each
2048-column block (host side) so the output DMA writes 640-byte
contiguous runs per partition; x is staged per 512-column sub-block
with 4KB contiguous per-partition DMA runs.
"""

import numpy as np
import ml_dtypes

import concourse.bass as bass
import concourse.mybir as mybir
import concourse.tile as tile
from concourse import bacc
from concourse.bass_utils import run_bass_kernel_spmd

BF16 = ml_dtypes.bfloat16
FP8 = ml_dtypes.float8_e4m3
F32 = np.float32

N_CORES = 8
B = 65536
S = B // N_CORES          # batch rows per core
IN_F = 784
NT = 7                    # h1 feature tiles (896 rows = 784 + o6 pad)
NXT = 8                   # x ext tiles ([ones + 127 payload] each)
PADL = 56                 # edge replication on each side of the perm axis
NPAY = IN_F + 2 * PADL    # payload rows (896)
H2 = 128
NCLS = 10
NSMX = 16                 # layer-3 batch tiles per softmax group
NGRP = S // (NSMX * 128)  # softmax groups == out-DMA blocks (4)
BLKC = S // NGRP          # batch columns per block (2048)
NSB = S // 512            # 512-col sub-blocks per core (16)

SW = 32.0                 # fp8 weight pre-scale; h1 at scale SW, h2 at SW*SW

WINDOW, STRIPES, STEP = 10, 5, 3

_CACHE = {}


def _butterfly_mask(out_f, in_f, window=WINDOW, stripes=STRIPES, step=STEP):
    i = np.arange(out_f)[:, None]
    j = np.arange(in_f)[None, :]
    jc = (i * in_f) // out_f
    band = np.abs(j - jc) <= window
    period = max(in_f // stripes, 1)
    stripe = ((j - jc) % period) < step
    return (band | stripe).astype(np.float32)


def _feat_perm():
    """Permutation sorting features by residue mod 156: makes the masked
    W1 circularly banded with half-bandwidth 55."""
    idx = np.arange(IN_F)
    return idx[np.lexsort((idx // 156, idx % 156))]


def _ext_pos():
    """Payload slot q -> permuted feature position (wrap-replicated)."""
    return np.concatenate(
        [np.arange(IN_F - PADL, IN_F), np.arange(IN_F), np.arange(PADL)]
    )


def _pin_act_tables(arch):
    # The act-table chooser greedily picks the first table containing each
    # function, so Relu/Exp/Ln would thrash between exp_and_others and
    # natural_log (1.3us ACT_TABLE_LOAD per swap).  Emptying those two
    # sets in the cached dict (indices stay stable for walrus's ID remap)
    # forces all three onto natural_log_exp_and_others -> a single table
    # load for the whole kernel.
    from concourse.hw_specs import get_activation_tables

    tabs = get_activation_tables(arch)
    if "natural_log_exp_and_others" in tabs:
        for name in ("exp_and_others", "natural_log"):
            if name in tabs:
                tabs[name].clear()


def _build_nc():
    nc = bacc.Bacc("TRN2", target_bir_lowering=False, debug=False, num_devices=N_CORES)
    _pin_act_tables(nc.m.arch)

    # host-side layouts are pre-arranged so every DMA is contiguous per
    # partition and every matmul operand is a direct SBUF slice.
    xe = nc.dram_tensor("xe", [NSB, 128, NXT, 512], mybir.dt.float8e4, kind="ExternalInput")
    w1e = nc.dram_tensor("w1e", [128, NT * 2 * 128], mybir.dt.float8e4, kind="ExternalInput")
    w2q = nc.dram_tensor("w2q", [128, NT * H2], mybir.dt.float8e4, kind="ExternalInput")
    w3q = nc.dram_tensor("w3q", [H2, NCLS], mybir.dt.bfloat16, kind="ExternalInput")
    bias = nc.dram_tensor("bias", [128, 1 + NCLS], mybir.dt.float32, kind="ExternalInput")
    out = nc.dram_tensor("out", [S, NCLS], mybir.dt.float32, kind="ExternalOutput")

    Relu = mybir.ActivationFunctionType.Relu
    Exp = mybir.ActivationFunctionType.Exp
    Ln = mybir.ActivationFunctionType.Ln
    X = mybir.AxisListType.X
    DR = mybir.MatmulPerfMode.DoubleRow
    ADD = mybir.AluOpType.add
    MAX = mybir.AluOpType.max
    MULT = mybir.AluOpType.mult

    with tile.TileContext(nc) as tc:
        with (
            tc.tile_pool(name="consts", bufs=1) as consts,
            tc.tile_pool(name="spool", bufs=3) as spool,
            tc.tile_pool(name="ps1", bufs=7, space="PSUM") as ps1,
            tc.tile_pool(name="ps2", bufs=1, space="PSUM") as ps2,
        ):
            # PE warm-up: dummy matmuls during the initial DMA wait flip the
            # HAM clock gate toward full rate before the real matmuls arrive.
            warm = consts.tile([128, 256], mybir.dt.float8e4)
            nc.gpsimd.memset(warm[:], 0.0)
            warm_ps = ps2.tile([128, 256], mybir.dt.float32, tag="ps2")
            for i in range(10):
                nc.tensor.matmul(
                    warm_ps[:],
                    warm[:, 0:128],
                    warm[:],
                    start=(i == 0),
                    stop=(i == 9),
                    skip_group_check=True,
                )
            # Scalar activation-table warm: loads the combined Exp/Ln/Relu
            # table during the DMA wait.
            scr = consts.tile([128, 2], mybir.dt.float32)
            nc.gpsimd.memset(scr[:], 1.0)
            scr2 = consts.tile([128, 2], mybir.dt.float32)
            nc.scalar.activation(scr2[:, 0:1], scr[:, 0:1], Exp)
            nc.scalar.activation(scr2[:, 1:2], scr[:, 1:2], Ln)

            # weights first (small), then the first 512-col x sub-block so
            # compute starts right after the DMA queue opens.
            w1_sb = consts.tile([128, NT, 2, 128], mybir.dt.float8e4)
            nc.sync.dma_start(
                w1_sb[:], w1e.rearrange("p (o t oi) -> p o t oi", o=NT, t=2)
            )
            xt_all = consts.tile([128, NSB, NXT, 512], mybir.dt.float8e4)
            nc.sync.dma_start(xt_all[:, 0, 0:2], xe[0, :, 0:2])
            nc.sync.dma_start(xt_all[:, 0, 2:NXT], xe[0, :, 2:NXT])
            w2_sb = consts.tile([128, NT, H2], mybir.dt.float8e4)
            nc.sync.dma_start(w2_sb[:], w2q.rearrange("p (kt o) -> p kt o", kt=NT))
            w3_sb = consts.tile([128, NCLS], mybir.dt.bfloat16)
            nc.sync.dma_start(w3_sb[:], w3q[:, :])
            bias_sb = consts.tile([128, 1 + NCLS], mybir.dt.float32)
            nc.sync.dma_start(bias_sb[:], bias[:, :])
            b2_sb = bias_sb[:, 0:1]
            b3_sb = bias_sb[:, 1:]

            # remaining sub-blocks: one DMA each, 4KB contiguous
            # per-partition destination runs
            for sb in range(1, NSB):
                nc.sync.dma_start(xt_all[:, sb], xe[sb])

            # persistent whole-shard activations
            h1_all = consts.tile([128, NT, S], mybir.dt.float8e4)
            h2_all = consts.tile([128, S], mybir.dt.bfloat16)

            def l2_mm(ns_p):
                # h2 psum = SW^2 * (h1 @ W2m.T) over the 896 permuted rows
                ps_l2 = ps2.tile([128, 512], mybir.dt.float32, tag="ps2")
                for p in range(3):
                    nc.tensor.matmul(
                        ps_l2[:],
                        w2_sb[:, 2 * p : 2 * p + 2, :],
                        h1_all[:, 2 * p : 2 * p + 2, ns_p],
                        start=(p == 0),
                        stop=False,
                        perf_mode=DR,
                    )
                nc.tensor.matmul(
                    ps_l2[:],
                    w2_sb[:, 6, :],
                    h1_all[:, 6, ns_p],
                    start=False,
                    stop=True,
                )
                return ps_l2

            def l2_evac(ps_prev, ns_prev):
                # h2 stored at scale SW^2; relu(psum + SW^2*b2) on Scalar
                nc.scalar.activation(
                    h2_all[:, ns_prev], ps_prev[:], Relu,
                    bias=b2_sb[:, 0:1], scale=1.0,
                )

            def l3_mm(ps_l, g, bt_lo, bt_hi):
                for bt in range(bt_lo, bt_hi):
                    bt_abs = g * NSMX + bt
                    nc.tensor.matmul(
                        ps_l[:, bt, :],
                        h2_all[:, bt_abs * 128 : (bt_abs + 1) * 128],
                        w3_sb[:, :],
                        start=(bt == 0),
                        stop=(bt == NSMX - 1),
                        skip_group_check=True,
                    )

            def l3_softmax(ps_l, g, bt_lo, bt_hi):
                # z = logits + b3 = psum / SW^2 + b3; |z| is O(5) so the
                # max-subtraction is skipped (exp safe in fp32).
                n = bt_hi - bt_lo
                pz = ps_l[:, bt_lo:bt_hi, :]
                z = spool.tile([128, n, NCLS], mybir.dt.float32, tag="z")
                nc.vector.scalar_tensor_tensor(
                    z[:],
                    pz,
                    1.0 / (SW * SW),
                    b3_sb[:, None, :].to_broadcast((128, n, NCLS)),
                    MULT,
                    ADD,
                )
                e = spool.tile([128, n, NCLS], mybir.dt.float32, tag="e")
                nc.scalar.activation(e[:], z[:], Exp)
                se = spool.tile([128, n], mybir.dt.float32, tag="se")
                nc.vector.reduce_sum(se[:], e[:], axis=X)
                lse = spool.tile([128, n], mybir.dt.float32, tag="lse")
                nc.scalar.activation(lse[:], se[:], Ln)
                nc.vector.tensor_sub(
                    e[:], z[:], lse[:, :, None].to_broadcast((128, n, NCLS))
                )
                # batch inside the block is host-permuted so partition p owns
                # 16 consecutive output rows -> 640B contiguous runs
                nc.sync.dma_start(
                    out[g * NSMX * 128 : (g + 1) * NSMX * 128, :].rearrange(
                        "(p bt) c -> p bt c", p=128
                    )[:, bt_lo:bt_hi],
                    e[:],
                )

            def do_l3(g):
                ps_l = ps2.tile([128, NSMX, NCLS], mybir.dt.float32, tag="ps2")
                l3_mm(ps_l, g, 0, NSMX)
                l3_softmax(ps_l, g, 0, NSMX)

            # Layer 2 for sub-block nb is emitted one iteration later (during
            # nb+1's layer 1) so its matmuls never head-block the in-order PE
            # queue while nb's h1 evacuations drain.
            pending = None
            for nb in range(NSB):
                # ---- layer 1: banded fp8; one uniform DR matmul per
                # o-tile, one psum bank each ----
                pss = {}
                for o in range(NT):
                    ps = ps1.tile([128, 512], mybir.dt.float32, tag="ps1")
                    pss[o] = ps
                    nc.tensor.matmul(
                        ps[:],
                        w1_sb[:, o, :, :],
                        xt_all[:, nb, o : o + 2, :],
                        start=True,
                        stop=True,
                        perf_mode=DR,
                    )
                # delayed layer-2 matmuls for the previous sub-block
                ps_l2 = None
                if pending is not None:
                    ns_p, nb_p = pending
                    ps_l2 = l2_mm(ns_p)
                # psum already includes SW*b1 (ones-row); h1 = relu(psum).
                # Evacuations alternate between the Vector and Scalar engines.
                ns = slice(nb * 512, (nb + 1) * 512)
                for o in range(NT):
                    h1_dst = h1_all[:, o, ns]
                    on_vec = o in (0, 2, 4) if o < 6 else nb % 2 == 0
                    if on_vec:
                        nc.vector.tensor_scalar_max(h1_dst, pss[o][:], 0.0)
                    else:
                        nc.scalar.activation(h1_dst, pss[o][:], Relu)
                if ps_l2 is not None:
                    l2_evac(ps_l2, ns_p)
                    if nb_p % (BLKC // 512) == BLKC // 512 - 1:
                        do_l3(nb_p // (BLKC // 512))
                pending = (ns, nb)

            # flush: final sub-block's layer 2 + layer 3
            ns_p, nb_p = pending
            ps_l2 = l2_mm(ns_p)
            l2_evac(ps_l2, ns_p)
            do_l3(NGRP - 1)

    return nc


def _block_perm():
    """Within each 2048-column block, shard position bt*128+p processes
    original row p*16+bt (so the output tile is DMA-contiguous)."""
    return np.arange(BLKC).reshape(128, NSMX).T.ravel()


def _prep_inputs(x, W1, b1, W2, b2, W3, b3):
    m1 = _butterfly_mask(IN_F, IN_F)
    m2 = _butterfly_mask(H2, IN_F)
    m3 = _butterfly_mask(NCLS, H2)
    P = _feat_perm()
    ep = _ext_pos()

    # w1: [in 784, out 784] masked, scaled by SW, both axes permuted.
    # Stationary per o-tile = ext tiles (o, o+1) in DR layout [p, o, t, oi];
    # tile rows: row 0 = ones-row (carries SW*b1 in tile o, zero in o+1),
    # rows 1..127 = payload slots q = 127*(o+t) + row - 1.
    w1p = ((np.asarray(W1, F32) * m1).T * SW)[np.ix_(P, P)]
    w1pay = np.zeros((NXT * 127, IN_F), dtype=F32)
    w1pay[:NPAY] = w1p[ep]
    b1p = (np.asarray(b1, F32) * SW)[P]
    w1el = np.zeros((128, NT, 2, 128), dtype=F32)
    for o in range(NT):
        ncols = min(IN_F - 128 * o, 128)
        cols = slice(128 * o, 128 * o + ncols)
        for t in range(2):
            pay = w1pay[127 * (o + t) : 127 * (o + t + 1), cols]  # [127, ncols]
            w1el[1:, o, t, :ncols] = pay
        w1el[0, o, 0, :ncols] = b1p[cols]
    w1el = np.ascontiguousarray(w1el.reshape(128, NT * 2 * 128)).astype(FP8)

    # w2: rows = permuted h1 features, zero-padded to 896; [p, kt, o] layout
    w2t = np.zeros((NT * 128, H2), dtype=F32)
    w2t[:IN_F] = ((np.asarray(W2, F32) * m2).T * SW)[P]
    w2l = np.ascontiguousarray(
        w2t.reshape(NT, 128, H2).transpose(1, 0, 2).reshape(128, NT * H2)
    ).astype(FP8)

    w3l = ((np.asarray(W3, F32) * m3).T).astype(BF16).copy()

    # bias pack [128, 1 + 10] f32: b2 scaled by SW^2; b3 broadcast.
    biasl = np.zeros((128, 1 + NCLS), F32)
    biasl[:, 0] = np.asarray(b2, F32) * (SW * SW)
    biasl[:, 1:] = np.asarray(b3, F32)[None, :]
    biasl = np.ascontiguousarray(biasl)

    # x: [B, 784] -> fp8, feature-permuted, payload+ones ext tiles,
    # batch permuted within each 2048-col block
    perm = _block_perm()
    full_perm = np.concatenate(
        [c * S + g * BLKC + perm for c in range(N_CORES) for g in range(NGRP)]
    )
    xT = np.asarray(x, F32).T.astype(FP8)[:, full_perm]
    xp = xT[P][ep]                       # [896, B] payload rows
    xe8 = np.zeros((NXT, 128, B), dtype=FP8)
    xe8[:, 0, :] = np.float32(1.0)
    pay = np.zeros((NXT * 127, B), dtype=FP8)
    pay[:NPAY] = xp
    xe8[:, 1:, :] = pay.reshape(NXT, 127, B)

    in_maps = []
    for c in range(N_CORES):
        xc = xe8[:, :, c * S : (c + 1) * S]
        # [NXT, 128, S] -> [NSB, 128, NXT, 512]: 4KB contiguous runs
        xc = np.ascontiguousarray(
            xc.reshape(NXT, 128, NSB, 512).transpose(2, 1, 0, 3)
        )
        in_maps.append(
            {
                "xe": xc,
                "w1e": w1el,
                "w2q": w2l,
                "w3q": w3l,
                "bias": biasl,
            }
        )
    return in_maps


def _run(inputs, trace=False, **run_kwargs):
    if "nc" not in _CACHE:
        nc = _build_nc()
        nc.finalize()
        _CACHE["nc"] = nc
    nc = _CACHE["nc"]
    in_maps = _prep_inputs(**inputs)
    res = run_bass_kernel_spmd(
        nc,
        in_maps,
        core_ids=list(range(N_CORES)),
        trace=trace,
        **run_kwargs,
    )
    out = np.concatenate([r["out"] for r in res.results], axis=0)
    return out, res


def kernel(**inputs):
    out, _ = _run(inputs, trace=False)
    return out


# revision 16
# speedup vs baseline: 1.1283x; 1.1283x over previous
"""ButterflyMlp Trainium2 kernel.

Reference computation (B=65536):
    h1 = relu(x @ (W1*m1).T + b1)          # [B, 784]
    h2 = relu(h1 @ (W2*m2).T + b2)         # [B, 128]
    logits = h2 @ (W3*m3).T + b3           # [B, 10]
    out = log_softmax(logits, axis=1)

Strategy: pure data parallel over 8 NeuronCores (batch sharded 8192/core,
masked weights replicated), activations kept in transposed [features,
batch] layout on-chip.

Key trick vs a dense L1: the butterfly mask for the square 784x784 layer
is nonzero only where (i - o) mod 156 is in {0,1,2} (stripes) or
|i - o| <= 10 (band).  Sorting BOTH feature axes by residue mod 156
maps every nonzero into a circular band of +-55 positions around the
diagonal.  The input features are laid out as 8 ext tiles of
[1 ones-row + 127 payload rows] (payload = the 784 permuted features
edge-replicated by 56 on each side); each 128-wide output tile of W1
then contracts exactly ext tiles (o, o+1), so layer 1 is 7 uniform
DoubleRow fp8 matmuls (K=256) per 512-column batch sub-block instead
of 21 DR + packed-tail passes of a dense kernel.  b1 rides on the
ones-rows, so the PSUM->SBUF evacuations are pure relu and can be
merged across o-tiles: the seven L1 psums live in two [128,2,512]
pairs + one [128,3,512] triple, giving 3 wide relu evacuations + 1 h2
evacuation per sub-block, split across the Scalar and Vector engines.

Layer 2 contracts the 896 permuted h1 rows as 3 DR chunks + 1 plain
K=128 tail (rows 784..895 of W2 are zero, so the h1 pad content is
irrelevant).  The Scalar engine's activation tables are pinned to
natural_log_exp_and_others (relu+exp+ln in one table) so only a single
ACT_TABLE_LOAD ever runs.  log_softmax skips the max-subtraction
(logits are O(5), far from fp32 exp overflow).

The masked weights are pre-scaled by 32 (h1 stored at scale 32, h2 at
1024) to keep fp8 values in the normal range; the scales fold back into
the relu / softmax stages.  The batch is permuted inside each
2048-column block (host side) so the output DMA writes 640-byte
contiguous runs per partition; x is staged per 512-column sub-block
with 4KB contiguous per-partition DMA runs.
"""

import numpy as np
import ml_dtypes

import concourse.bass as bass
import concourse.mybir as mybir
import concourse.tile as tile
from concourse import bacc
from concourse.bass_utils import run_bass_kernel_spmd

BF16 = ml_dtypes.bfloat16
FP8 = ml_dtypes.float8_e4m3
F32 = np.float32

N_CORES = 8
B = 65536
S = B // N_CORES          # batch rows per core
IN_F = 784
NT = 7                    # h1 feature tiles (896 rows = 784 + o6 pad)
NXT = 8                   # x ext tiles ([ones + 127 payload] each)
PADL = 56                 # edge replication on each side of the perm axis
NPAY = IN_F + 2 * PADL    # payload rows (896)
H2 = 128
NCLS = 10
NSMX = 16                 # layer-3 batch tiles per softmax group
NGRP = S // (NSMX * 128)  # softmax groups == out-DMA blocks (4)
BLKC = S // NGRP          # batch columns per block (2048)
NSB = S // 512            # 512-col sub-blocks per core (16)

SW = 32.0                 # fp8 weight pre-scale; h1 at scale SW, h2 at SW*SW

WINDOW, STRIPES, STEP = 10, 5, 3

_CACHE = {}


def _butterfly_mask(out_f, in_f, window=WINDOW, stripes=STRIPES, step=STEP):
    i = np.arange(out_f)[:, None]
    j = np.arange(in_f)[None, :]
    jc = (i * in_f) // out_f
    band = np.abs(j - jc) <= window
    period = max(in_f // stripes, 1)
    stripe = ((j - jc) % period) < step
    return (band | stripe).astype(np.float32)


def _feat_perm():
    """Permutation sorting features by residue mod 156: makes the masked
    W1 circularly banded with half-bandwidth 55."""
    idx = np.arange(IN_F)
    return idx[np.lexsort((idx // 156, idx % 156))]


def _ext_pos():
    """Payload slot q -> permuted feature position (wrap-replicated)."""
    return np.concatenate(
        [np.arange(IN_F - PADL, IN_F), np.arange(IN_F), np.arange(PADL)]
    )


def _pin_act_tables(arch):
    # The act-table chooser greedily picks the first table containing each
    # function, so Relu/Exp/Ln would thrash between exp_and_others and
    # natural_log (1.3us ACT_TABLE_LOAD per swap).  Emptying those two
    # sets in the cached dict (indices stay stable for walrus's ID remap)
    # forces all three onto natural_log_exp_and_others -> a single table
    # load for the whole kernel.
    from concourse.hw_specs import get_activation_tables

    tabs = get_activation_tables(arch)
    if "natural_log_exp_and_others" in tabs:
        for name in ("exp_and_others", "natural_log"):
            if name in tabs:
                tabs[name].clear()


def _build_nc():
    nc = bacc.Bacc("TRN2", target_bir_lowering=False, debug=False, num_devices=N_CORES)
    _pin_act_tables(nc.m.arch)

    # host-side layouts are pre-arranged so every DMA is contiguous per
    # partition and every matmul operand is a direct SBUF slice.
    xe = nc.dram_tensor("xe", [NSB, 128, NXT, 512], mybir.dt.float8e4, kind="ExternalInput")
    w1e = nc.dram_tensor("w1e", [128, NT * 2 * 128], mybir.dt.float8e4, kind="ExternalInput")
    w2q = nc.dram_tensor("w2q", [128, NT * H2], mybir.dt.float8e4, kind="ExternalInput")
    w3q = nc.dram_tensor("w3q", [H2, NCLS], mybir.dt.bfloat16, kind="ExternalInput")
    bias = nc.dram_tensor("bias", [128, 1 + NCLS], mybir.dt.float32, kind="ExternalInput")
    out = nc.dram_tensor("out", [S, NCLS], mybir.dt.float32, kind="ExternalOutput")

    Relu = mybir.ActivationFunctionType.Relu
    Exp = mybir.ActivationFunctionType.Exp
    Ln = mybir.ActivationFunctionType.Ln
    X = mybir.AxisListType.X
    DR = mybir.MatmulPerfMode.DoubleRow
    ADD = mybir.AluOpType.add
    MAX = mybir.AluOpType.max
    MULT = mybir.AluOpType.mult

    with tile.TileContext(nc) as tc:
        with (
            tc.tile_pool(name="consts", bufs=1) as consts,
            tc.tile_pool(name="spool", bufs=3) as spool,
            tc.tile_pool(name="ps1", bufs=7, space="PSUM") as ps1,
            tc.tile_pool(name="ps2", bufs=1, space="PSUM") as ps2,
        ):
            # PE warm-up: dummy matmuls during the initial DMA wait flip the
            # HAM clock gate toward full rate before the real matmuls arrive.
            warm = consts.tile([128, 512], mybir.dt.float8e4)
            nc.gpsimd.memset(warm[:], 0.0)
            warm_ps = ps2.tile([128, 512], mybir.dt.float32, tag="ps2")
            for i in range(14):
                nc.tensor.matmul(
                    warm_ps[:],
                    warm[:, 0:128],
                    warm[:],
                    start=(i == 0),
                    stop=(i == 13),
                    skip_group_check=True,
                )
            # Scalar activation-table warm: loads the combined Exp/Ln/Relu
            # table during the DMA wait.
            scr = consts.tile([128, 2], mybir.dt.float32)
            nc.gpsimd.memset(scr[:], 1.0)
            scr2 = consts.tile([128, 2], mybir.dt.float32)
            nc.scalar.activation(scr2[:, 0:1], scr[:, 0:1], Exp)
            nc.scalar.activation(scr2[:, 1:2], scr[:, 1:2], Ln)

            # weights first (small), then the first 512-col x sub-block so
            # compute starts right after the DMA queue opens.
            w1_sb = consts.tile([128, NT, 2, 128], mybir.dt.float8e4)
            nc.sync.dma_start(
                w1_sb[:], w1e.rearrange("p (o t oi) -> p o t oi", o=NT, t=2)
            )
            xt_all = consts.tile([128, NSB, NXT, 512], mybir.dt.float8e4)
            nc.sync.dma_start(xt_all[:, 0, 0:2], xe[0, :, 0:2])
            nc.sync.dma_start(xt_all[:, 0, 2:NXT], xe[0, :, 2:NXT])
            w2_sb = consts.tile([128, NT, H2], mybir.dt.float8e4)
            nc.sync.dma_start(w2_sb[:], w2q.rearrange("p (kt o) -> p kt o", kt=NT))
            w3_sb = consts.tile([128, NCLS], mybir.dt.bfloat16)
            nc.sync.dma_start(w3_sb[:], w3q[:, :])
            bias_sb = consts.tile([128, 1 + NCLS], mybir.dt.float32)
            nc.sync.dma_start(bias_sb[:], bias[:, :])
            b2_sb = bias_sb[:, 0:1]
            b3_sb = bias_sb[:, 1:]

            # remaining sub-blocks: one DMA each, 4KB contiguous
            # per-partition destination runs
            for sb in range(1, NSB):
                nc.sync.dma_start(xt_all[:, sb], xe[sb])

            # persistent whole-shard activations
            h1_all = consts.tile([128, NT, S], mybir.dt.float8e4)
            h2_all = consts.tile([128, S], mybir.dt.bfloat16)

            def l2_mm(ns_p):
                # h2 psum = SW^2 * (h1 @ W2m.T) over the 896 permuted rows
                ps_l2 = ps2.tile([128, 512], mybir.dt.float32, tag="ps2")
                for p in range(3):
                    nc.tensor.matmul(
                        ps_l2[:],
                        w2_sb[:, 2 * p : 2 * p + 2, :],
                        h1_all[:, 2 * p : 2 * p + 2, ns_p],
                        start=(p == 0),
                        stop=False,
                        perf_mode=DR,
                    )
                nc.tensor.matmul(
                    ps_l2[:],
                    w2_sb[:, 6, :],
                    h1_all[:, 6, ns_p],
                    start=False,
                    stop=True,
                )
                return ps_l2

            def l2_evac(ps_prev, ns_prev):
                # h2 stored at scale SW^2; relu(psum + SW^2*b2) on Scalar
                nc.scalar.activation(
                    h2_all[:, ns_prev], ps_prev[:], Relu,
                    bias=b2_sb[:, 0:1], scale=1.0,
                )

            def l3_mm(ps_l, g, bt_lo, bt_hi):
                for bt in range(bt_lo, bt_hi):
                    bt_abs = g * NSMX + bt
                    nc.tensor.matmul(
                        ps_l[:, bt, :],
                        h2_all[:, bt_abs * 128 : (bt_abs + 1) * 128],
                        w3_sb[:, :],
                        start=(bt == 0),
                        stop=(bt == NSMX - 1),
                        skip_group_check=True,
                    )

            def l3_softmax(ps_l, g, bt_lo, bt_hi):
                # z = logits + b3 = psum / SW^2 + b3; |z| is O(5) so the
                # max-subtraction is skipped (exp safe in fp32).
                n = bt_hi - bt_lo
                pz = ps_l[:, bt_lo:bt_hi, :]
                z = spool.tile([128, n, NCLS], mybir.dt.float32, tag="z")
                nc.vector.scalar_tensor_tensor(
                    z[:],
                    pz,
                    1.0 / (SW * SW),
                    b3_sb[:, None, :].to_broadcast((128, n, NCLS)),
                    MULT,
                    ADD,
                )
                e = spool.tile([128, n, NCLS], mybir.dt.float32, tag="e")
                nc.scalar.activation(e[:], z[:], Exp)
                se = spool.tile([128, n], mybir.dt.float32, tag="se")
                nc.vector.reduce_sum(se[:], e[:], axis=X)
                lse = spool.tile([128, n], mybir.dt.float32, tag="lse")
                nc.scalar.activation(lse[:], se[:], Ln)
                nc.vector.tensor_sub(
                    e[:], z[:], lse[:, :, None].to_broadcast((128, n, NCLS))
                )
                # batch inside the block is host-permuted so partition p owns
                # 16 consecutive output rows -> 640B contiguous runs
                nc.sync.dma_start(
                    out[g * NSMX * 128 : (g + 1) * NSMX * 128, :].rearrange(
                        "(p bt) c -> p bt c", p=128
                    )[:, bt_lo:bt_hi],
                    e[:],
                )

            def do_l3(g):
                ps_l = ps2.tile([128, NSMX, NCLS], mybir.dt.float32, tag="ps2")
                l3_mm(ps_l, g, 0, NSMX)
                l3_softmax(ps_l, g, 0, NSMX)

            # Layer 2 for sub-block nb is emitted one iteration later (during
            # nb+1's layer 1) so its matmuls never head-block the in-order PE
            # queue while nb's h1 evacuations drain.
            pending = None
            for nb in range(NSB):
                # ---- layer 1: banded fp8; one uniform DR matmul per
                # o-tile, one psum bank each ----
                pss = {}
                for o in range(NT):
                    ps = ps1.tile([128, 512], mybir.dt.float32, tag="ps1")
                    pss[o] = ps
                    nc.tensor.matmul(
                        ps[:],
                        w1_sb[:, o, :, :],
                        xt_all[:, nb, o : o + 2, :],
                        start=True,
                        stop=True,
                        perf_mode=DR,
                    )
                # delayed layer-2 matmuls for the previous sub-block
                ps_l2 = None
                if pending is not None:
                    ns_p, nb_p = pending
                    ps_l2 = l2_mm(ns_p)
                # psum already includes SW*b1 (ones-row); h1 = relu(psum).
                # Evacuations alternate between the Vector and Scalar engines.
                ns = slice(nb * 512, (nb + 1) * 512)
                for o in range(NT):
                    h1_dst = h1_all[:, o, ns]
                    on_vec = o in (0, 2, 4) if o < 6 else nb % 2 == 0
                    if on_vec:
                        nc.vector.tensor_scalar_max(h1_dst, pss[o][:], 0.0)
                    else:
                        nc.scalar.activation(h1_dst, pss[o][:], Relu)
                if ps_l2 is not None:
                    l2_evac(ps_l2, ns_p)
                    if nb_p % (BLKC // 512) == BLKC // 512 - 1:
                        do_l3(nb_p // (BLKC // 512))
                pending = (ns, nb)

            # flush: final sub-block's layer 2 + layer 3.  Zero-weight
            # dummy matmuls interleave with the drain so the HAM clock
            # stays at full rate through the tail and teardown.
            def dummies(n):
                dp = ps1.tile([128, 512], mybir.dt.float32, tag="ps1")
                for i in range(n):
                    nc.tensor.matmul(
                        dp[:],
                        warm[:, 0:128],
                        warm[:],
                        start=(i == 0),
                        stop=(i == n - 1),
                        skip_group_check=True,
                    )

            ns_p, nb_p = pending
            dummies(4)
            ps_l2 = l2_mm(ns_p)
            l2_evac(ps_l2, ns_p)
            dummies(4)
            do_l3(NGRP - 1)
            dummies(16)

    return nc


def _block_perm():
    """Within each 2048-column block, shard position bt*128+p processes
    original row p*16+bt (so the output tile is DMA-contiguous)."""
    return np.arange(BLKC).reshape(128, NSMX).T.ravel()


def _prep_inputs(x, W1, b1, W2, b2, W3, b3):
    m1 = _butterfly_mask(IN_F, IN_F)
    m2 = _butterfly_mask(H2, IN_F)
    m3 = _butterfly_mask(NCLS, H2)
    P = _feat_perm()
    ep = _ext_pos()

    # w1: [in 784, out 784] masked, scaled by SW, both axes permuted.
    # Stationary per o-tile = ext tiles (o, o+1) in DR layout [p, o, t, oi];
    # tile rows: row 0 = ones-row (carries SW*b1 in tile o, zero in o+1),
    # rows 1..127 = payload slots q = 127*(o+t) + row - 1.
    w1p = ((np.asarray(W1, F32) * m1).T * SW)[np.ix_(P, P)]
    w1pay = np.zeros((NXT * 127, IN_F), dtype=F32)
    w1pay[:NPAY] = w1p[ep]
    b1p = (np.asarray(b1, F32) * SW)[P]
    w1el = np.zeros((128, NT, 2, 128), dtype=F32)
    for o in range(NT):
        ncols = min(IN_F - 128 * o, 128)
        cols = slice(128 * o, 128 * o + ncols)
        for t in range(2):
            pay = w1pay[127 * (o + t) : 127 * (o + t + 1), cols]  # [127, ncols]
            w1el[1:, o, t, :ncols] = pay
        w1el[0, o, 0, :ncols] = b1p[cols]
    w1el = np.ascontiguousarray(w1el.reshape(128, NT * 2 * 128)).astype(FP8)

    # w2: rows = permuted h1 features, zero-padded to 896; [p, kt, o] layout
    w2t = np.zeros((NT * 128, H2), dtype=F32)
    w2t[:IN_F] = ((np.asarray(W2, F32) * m2).T * SW)[P]
    w2l = np.ascontiguousarray(
        w2t.reshape(NT, 128, H2).transpose(1, 0, 2).reshape(128, NT * H2)
    ).astype(FP8)

    w3l = ((np.asarray(W3, F32) * m3).T).astype(BF16).copy()

    # bias pack [128, 1 + 10] f32: b2 scaled by SW^2; b3 broadcast.
    biasl = np.zeros((128, 1 + NCLS), F32)
    biasl[:, 0] = np.asarray(b2, F32) * (SW * SW)
    biasl[:, 1:] = np.asarray(b3, F32)[None, :]
    biasl = np.ascontiguousarray(biasl)

    # x: [B, 784] -> fp8, feature-permuted, payload+ones ext tiles,
    # batch permuted within each 2048-col block
    perm = _block_perm()
    full_perm = np.concatenate(
        [c * S + g * BLKC + perm for c in range(N_CORES) for g in range(NGRP)]
    )
    xT = np.asarray(x, F32).T.astype(FP8)[:, full_perm]
    xp = xT[P][ep]                       # [896, B] payload rows
    xe8 = np.zeros((NXT, 128, B), dtype=FP8)
    xe8[:, 0, :] = np.float32(1.0)
    pay = np.zeros((NXT * 127, B), dtype=FP8)
    pay[:NPAY] = xp
    xe8[:, 1:, :] = pay.reshape(NXT, 127, B)

    in_maps = []
    for c in range(N_CORES):
        xc = xe8[:, :, c * S : (c + 1) * S]
        # [NXT, 128, S] -> [NSB, 128, NXT, 512]: 4KB contiguous runs
        xc = np.ascontiguousarray(
            xc.reshape(NXT, 128, NSB, 512).transpose(2, 1, 0, 3)
        )
        in_maps.append(
            {
                "xe": xc,
                "w1e": w1el,
                "w2q": w2l,
                "w3q": w3l,
                "bias": biasl,
            }
        )
    return in_maps


def _run(inputs, trace=False, **run_kwargs):
    if "nc" not in _CACHE:
        nc = _build_nc()
        nc.finalize()
        _CACHE["nc"] = nc
    nc = _CACHE["nc"]
    in_maps = _prep_inputs(**inputs)
    res = run_bass_kernel_spmd(
        nc,
        in_maps,
        core_ids=list(range(N_CORES)),
        trace=trace,
        **run_kwargs,
    )
    out = np.concatenate([r["out"] for r in res.results], axis=0)
    return out, res


def kernel(**inputs):
    out, _ = _run(inputs, trace=False)
    return out


# revision 28
# speedup vs baseline: 1.2658x; 1.1218x over previous
"""ButterflyMlp Trainium2 kernel.

Reference computation (B=65536):
    h1 = relu(x @ (W1*m1).T + b1)          # [B, 784]
    h2 = relu(h1 @ (W2*m2).T + b2)         # [B, 128]
    logits = h2 @ (W3*m3).T + b3           # [B, 10]
    out = log_softmax(logits, axis=1)

Strategy: pure data parallel over 8 NeuronCores (batch sharded 8192/core,
masked weights replicated), activations kept in transposed [features,
batch] layout on-chip.

Key trick vs a dense L1: the butterfly mask for the square 784x784 layer
is nonzero only where (i - o) mod 156 is in {0,1,2} (stripes) or
|i - o| <= 10 (band).  Sorting BOTH feature axes by residue mod 156
maps every nonzero into a circular band of +-55 positions around the
diagonal.  The input features are laid out as 8 ext tiles of
[1 ones-row + 127 payload rows] (payload = the 784 permuted features
edge-replicated by 56 on each side); each 128-wide output tile of W1
then contracts exactly ext tiles (o, o+1), so layer 1 is 7 uniform
DoubleRow fp8 matmuls (K=256) per 512-column batch sub-block instead
of 21 DR + packed-tail passes of a dense kernel.  b1 rides on the
ones-rows, so the PSUM->SBUF evacuations are pure relu: per sub-block
the Vector engine takes o0/o2/o4 + the h2 evacuation, the Scalar
engine takes o1/o3/o5/o6 (the engines, not the PE, bound the steady
state at ~3.1us per sub-block).  Layer 2 for sub-block nb is emitted
during nb+1's layer 1 so its matmuls never head-block the in-order PE
queue, and each block's log-softmax chain is emitted in stages
(generator pump, 2 per iteration) interleaved with the evacuation
stream so the serial chain never head-blocks the engine queues; the
last block is split into micro-groups so the final chain is short.

Layer 2 contracts the 896 permuted h1 rows as 3 DR chunks + 1 plain
K=128 tail (rows 784..895 of W2 are zero, so the h1 pad content is
irrelevant).  The Scalar engine's activation tables are pinned to
natural_log_exp_and_others (relu+exp+ln in one table) so only a single
ACT_TABLE_LOAD ever runs.  log_softmax skips the max-subtraction
(logits are O(5), far from fp32 exp overflow).  Zero-weight dummy
matmuls warm the PE's HAM clock gate during the initial DMA wait and
hold it at full rate through the drain.

The masked weights are pre-scaled by 32 (h1 stored at scale 32, h2 at
1024) to keep fp8 values in the normal range; the scales fold back into
the relu / softmax stages.  The batch is permuted inside each
2048-column block (host side) so the output DMA writes 640-byte
contiguous runs per partition; x is staged per 512-column sub-block
with 4KB contiguous per-partition DMA runs.
"""

import numpy as np
import ml_dtypes

import concourse.mybir as mybir
import concourse.tile as tile
from concourse import bacc
from concourse.bass_utils import run_bass_kernel_spmd

BF16 = ml_dtypes.bfloat16
FP8 = ml_dtypes.float8_e4m3
F32 = np.float32

N_CORES = 8
B = 65536
S = B // N_CORES          # batch rows per core
IN_F = 784
NT = 7                    # h1 feature tiles (896 rows = 784 + o6 pad)
NXT = 8                   # x ext tiles ([ones + 127 payload] each)
PADL = 56                 # edge replication on each side of the perm axis
NPAY = IN_F + 2 * PADL    # payload rows (896)
H2 = 128
NCLS = 10
NSMX = 16                 # layer-3 batch tiles per softmax group
NGRP = S // (NSMX * 128)  # softmax groups == out-DMA blocks (4)
BLKC = S // NGRP          # batch columns per block (2048)
NSB = S // 512            # 512-col sub-blocks per core (16)

SW = 32.0                 # fp8 weight pre-scale; h1 at scale SW, h2 at SW*SW

WINDOW, STRIPES, STEP = 10, 5, 3

_CACHE = {}


def _butterfly_mask(out_f, in_f, window=WINDOW, stripes=STRIPES, step=STEP):
    i = np.arange(out_f)[:, None]
    j = np.arange(in_f)[None, :]
    jc = (i * in_f) // out_f
    band = np.abs(j - jc) <= window
    period = max(in_f // stripes, 1)
    stripe = ((j - jc) % period) < step
    return (band | stripe).astype(np.float32)


def _feat_perm():
    """Permutation sorting features by residue mod 156: makes the masked
    W1 circularly banded with half-bandwidth 55."""
    idx = np.arange(IN_F)
    return idx[np.lexsort((idx // 156, idx % 156))]


def _ext_pos():
    """Payload slot q -> permuted feature position (wrap-replicated)."""
    return np.concatenate(
        [np.arange(IN_F - PADL, IN_F), np.arange(IN_F), np.arange(PADL)]
    )


def _pin_act_tables(arch):
    # The act-table chooser greedily picks the first table containing each
    # function, so Relu/Exp/Ln would thrash between exp_and_others and
    # natural_log (1.3us ACT_TABLE_LOAD per swap).  Emptying those two
    # sets in the cached dict (indices stay stable for walrus's ID remap)
    # forces all three onto natural_log_exp_and_others -> a single table
    # load for the whole kernel.
    from concourse.hw_specs import get_activation_tables

    tabs = get_activation_tables(arch)
    if "natural_log_exp_and_others" in tabs:
        for name in ("exp_and_others", "natural_log"):
            if name in tabs:
                tabs[name].clear()


def _build_nc():
    nc = bacc.Bacc("TRN2", target_bir_lowering=False, debug=False, num_devices=N_CORES)
    _pin_act_tables(nc.m.arch)

    # host-side layouts are pre-arranged so every DMA is contiguous per
    # partition and every matmul operand is a direct SBUF slice.
    xe = nc.dram_tensor("xe", [NSB, 128, NXT, 512], mybir.dt.float8e4, kind="ExternalInput")
    w1e = nc.dram_tensor("w1e", [128, NT * 2 * 128], mybir.dt.float8e4, kind="ExternalInput")
    w2q = nc.dram_tensor("w2q", [128, NT * H2], mybir.dt.float8e4, kind="ExternalInput")
    w3q = nc.dram_tensor("w3q", [H2, NCLS], mybir.dt.bfloat16, kind="ExternalInput")
    bias = nc.dram_tensor("bias", [128, 1 + NCLS], mybir.dt.float32, kind="ExternalInput")
    out = nc.dram_tensor("out", [S, NCLS], mybir.dt.float32, kind="ExternalOutput")

    Relu = mybir.ActivationFunctionType.Relu
    Exp = mybir.ActivationFunctionType.Exp
    Ln = mybir.ActivationFunctionType.Ln
    X = mybir.AxisListType.X
    DR = mybir.MatmulPerfMode.DoubleRow
    ADD = mybir.AluOpType.add
    MAX = mybir.AluOpType.max
    MULT = mybir.AluOpType.mult

    with tile.TileContext(nc) as tc:
        with (
            tc.tile_pool(name="consts", bufs=1) as consts,
            tc.tile_pool(name="spool", bufs=3) as spool,
            tc.tile_pool(name="ps1", bufs=7, space="PSUM") as ps1,
            tc.tile_pool(name="ps2", bufs=1, space="PSUM") as ps2,
        ):
            # PE warm-up: dummy matmuls during the initial DMA wait flip the
            # HAM clock gate toward full rate before the real matmuls arrive.
            warm = consts.tile([128, 512], mybir.dt.float8e4)
            nc.gpsimd.memset(warm[:], 0.0)
            warm_ps = ps2.tile([128, 512], mybir.dt.float32, tag="ps2")
            for i in range(6):
                nc.tensor.matmul(
                    warm_ps[:],
                    warm[:, 0:128],
                    warm[:],
                    start=(i == 0),
                    stop=(i == 5),
                    skip_group_check=True,
                )
            # Scalar activation-table warm: loads the combined Exp/Ln/Relu
            # table during the DMA wait.
            scr = consts.tile([128, 2], mybir.dt.float32)
            nc.gpsimd.memset(scr[:], 1.0)
            scr2 = consts.tile([128, 2], mybir.dt.float32)
            nc.scalar.activation(scr2[:, 0:1], scr[:, 0:1], Exp)
            nc.scalar.activation(scr2[:, 1:2], scr[:, 1:2], Ln)

            # weights first (small), then the first 512-col x sub-block so
            # compute starts right after the DMA queue opens.
            w1_sb = consts.tile([128, NT, 2, 128], mybir.dt.float8e4)
            w1r = w1e.rearrange("p (o t oi) -> p o t oi", o=NT, t=2)
            nc.sync.dma_start(w1_sb[:, 0], w1r[:, 0])
            xt_all = consts.tile([128, NSB, NXT, 512], mybir.dt.float8e4)
            nc.sync.dma_start(xt_all[:, 0, 0:2], xe[0, :, 0:2])
            nc.sync.dma_start(w1_sb[:, 1:], w1r[:, 1:])
            nc.sync.dma_start(xt_all[:, 0, 2:NXT], xe[0, :, 2:NXT])
            bias_sb = consts.tile([128, 1 + NCLS], mybir.dt.float32)
            nc.sync.dma_start(bias_sb[:], bias[:, :])
            b2_sb = bias_sb[:, 0:1]
            b3_sb = bias_sb[:, 1:]
            nc.sync.dma_start(xt_all[:, 1], xe[1])
            w2_sb = consts.tile([128, NT, H2], mybir.dt.float8e4)
            nc.sync.dma_start(w2_sb[:], w2q.rearrange("p (kt o) -> p kt o", kt=NT))
            w3_sb = consts.tile([128, NCLS], mybir.dt.bfloat16)
            nc.sync.dma_start(w3_sb[:], w3q[:, :])

            # remaining sub-blocks: one DMA each, 4KB contiguous
            # per-partition destination runs
            for sb in range(2, NSB):
                nc.sync.dma_start(xt_all[:, sb], xe[sb])

            # persistent whole-shard activations
            h1_all = consts.tile([128, NT, S], mybir.dt.float8e4)
            h2_all = consts.tile([128, S], mybir.dt.bfloat16)

            def l2_mm(ns_p):
                # h2 psum = SW^2 * (h1 @ W2m.T) over the 896 permuted rows
                ps_l2 = ps2.tile([128, 512], mybir.dt.float32, tag="ps2")
                for p in range(3):
                    nc.tensor.matmul(
                        ps_l2[:],
                        w2_sb[:, 2 * p : 2 * p + 2, :],
                        h1_all[:, 2 * p : 2 * p + 2, ns_p],
                        start=(p == 0),
                        stop=False,
                        perf_mode=DR,
                    )
                nc.tensor.matmul(
                    ps_l2[:],
                    w2_sb[:, 6, :],
                    h1_all[:, 6, ns_p],
                    start=False,
                    stop=True,
                )
                return ps_l2

            def l2_evac(ps_prev, ns_prev, parity):
                # h2 stored at scale SW^2; relu(psum + SW^2*b2); alternates
                # engines counter-phase to the o6 evacuation
                nc.vector.tensor_scalar(
                    h2_all[:, ns_prev], ps_prev[:], b2_sb[:, 0:1], 0.0, ADD, MAX
                )

            def l3_chain(bt_lo, bt_hi):
                # L3 matmuls + log-softmax for global batch tiles
                # [bt_lo, bt_hi), yielded in stages so the caller can
                # interleave other engine work between the chain links.
                # z = logits + b3 = psum / SW^2 + b3; |z| is O(5) so the
                # max-subtraction is skipped (exp safe in fp32).
                n = bt_hi - bt_lo
                ps_l = ps1.tile([128, n, NCLS], mybir.dt.float32, tag="ps1")
                for bt in range(n):
                    bt_abs = bt_lo + bt
                    nc.tensor.matmul(
                        ps_l[:, bt, :],
                        h2_all[:, bt_abs * 128 : (bt_abs + 1) * 128],
                        w3_sb[:, :],
                        start=(bt == 0),
                        stop=(bt == n - 1),
                        skip_group_check=True,
                    )
                z = spool.tile([128, n, NCLS], mybir.dt.float32, tag="z")
                nc.vector.scalar_tensor_tensor(
                    z[:],
                    ps_l[:],
                    1.0 / (SW * SW),
                    b3_sb[:, None, :].to_broadcast((128, n, NCLS)),
                    MULT,
                    ADD,
                )
                yield
                e = spool.tile([128, n, NCLS], mybir.dt.float32, tag="e")
                nc.scalar.activation(e[:], z[:], Exp)
                yield
                se = spool.tile([128, n], mybir.dt.float32, tag="se")
                nc.vector.reduce_sum(se[:], e[:], axis=X)
                yield
                lse = spool.tile([128, n], mybir.dt.float32, tag="lse")
                nc.scalar.activation(lse[:], se[:], Ln)
                yield
                nc.gpsimd.tensor_sub(
                    e[:], z[:], lse[:, :, None].to_broadcast((128, n, NCLS))
                )
                # batch inside each block is host-permuted so partition p
                # owns 16 consecutive output rows -> contiguous runs
                g, lo = divmod(bt_lo, NSMX)
                nc.sync.dma_start(
                    out[g * NSMX * 128 : (g + 1) * NSMX * 128, :].rearrange(
                        "(p bt) c -> p bt c", p=128
                    )[:, lo : lo + n],
                    e[:],
                )

            # Layer 2 for sub-block nb is emitted one iteration later (during
            # nb+1's layer 1) so its matmuls never head-block the in-order PE
            # queue while nb's h1 evacuations drain.
            pending = None
            chains = []
            for nb in range(NSB):
                # ---- layer 1: banded fp8; one uniform DR matmul per
                # o-tile, one psum bank each ----
                pss = {}
                for o in range(NT):
                    ps = ps1.tile([128, 512], mybir.dt.float32, tag="ps1")
                    pss[o] = ps
                    nc.tensor.matmul(
                        ps[:],
                        w1_sb[:, o, :, :],
                        xt_all[:, nb, o : o + 2, :],
                        start=True,
                        stop=True,
                        perf_mode=DR,
                    )
                # delayed layer-2 matmuls for the previous sub-block
                ps_l2 = None
                if pending is not None:
                    ns_p, nb_p = pending
                    ps_l2 = l2_mm(ns_p)
                # psum already includes SW*b1 (ones-row); h1 = relu(psum).
                # Evacuations: (o0,o2,o4) on Vector, (o1,o3,o5,o6) on Scalar.
                ns = slice(nb * 512, (nb + 1) * 512)
                for o in range(NT):
                    h1_dst = h1_all[:, o, ns]
                    if o in (0, 2, 4):
                        nc.vector.tensor_scalar_max(h1_dst, pss[o][:], 0.0)
                    else:
                        nc.scalar.activation(h1_dst, pss[o][:], Relu)
                if ps_l2 is not None:
                    l2_evac(ps_l2, ns_p, nb_p)
                    # start a log-softmax chain once its h2 tiles are done:
                    # full groups for blocks 0-2, micro-groups at the tail
                    # ({12,13}, {14}, {15}) so the final chain is short
                    if nb_p in (3, 7, 11):
                        chains.append(l3_chain(nb_p * 4 - 12, nb_p * 4 + 4))
                    elif nb_p == 13:
                        chains.append(l3_chain(48, 56))
                    elif nb_p == 14:
                        chains.append(l3_chain(56, 60))
                # pump pending chains: two stages per iteration, interleaved
                # with the evac stream so chain waits never head-block queues
                for _ in range(2):
                    if chains:
                        try:
                            next(chains[0])
                        except StopIteration:
                            chains.pop(0)
                pending = (ns, nb)

            # flush: final sub-block's layer 2 + layer 3.  Zero-weight
            # dummy matmuls interleave with the drain so the HAM clock
            # stays at full rate through the tail and teardown.
            def dummies(n):
                dpt = ps1.tile([128, 512], mybir.dt.float32, tag="ps1")
                dp = dpt[:]
                for i in range(n):
                    nc.tensor.matmul(
                        dp,
                        warm[:, 0:128],
                        warm[:],
                        start=(i == 0),
                        stop=(i == n - 1),
                        skip_group_check=True,
                    )

            ns_p, nb_p = pending
            dummies(4)
            ps_l2 = l2_mm(ns_p)
            l2_evac(ps_l2, ns_p, nb_p)
            dummies(4)
            chains.append(l3_chain(60, 64))
            while chains:
                try:
                    next(chains[0])
                except StopIteration:
                    chains.pop(0)
            dummies(8)

    return nc


def _block_perm():
    """Within each 2048-column block, shard position bt*128+p processes
    original row p*16+bt (so the output tile is DMA-contiguous)."""
    return np.arange(BLKC).reshape(128, NSMX).T.ravel()


def _prep_inputs(x, W1, b1, W2, b2, W3, b3):
    m1 = _butterfly_mask(IN_F, IN_F)
    m2 = _butterfly_mask(H2, IN_F)
    m3 = _butterfly_mask(NCLS, H2)
    P = _feat_perm()
    ep = _ext_pos()

    # w1: [in 784, out 784] masked, scaled by SW, both axes permuted.
    # Stationary per o-tile = ext tiles (o, o+1) in DR layout [p, o, t, oi];
    # tile rows: row 0 = ones-row (carries SW*b1 in tile o, zero in o+1),
    # rows 1..127 = payload slots q = 127*(o+t) + row - 1.
    w1p = ((np.asarray(W1, F32) * m1).T * SW)[np.ix_(P, P)]
    w1pay = np.zeros((NXT * 127, IN_F), dtype=F32)
    w1pay[:NPAY] = w1p[ep]
    b1p = (np.asarray(b1, F32) * SW)[P]
    w1el = np.zeros((128, NT, 2, 128), dtype=F32)
    for o in range(NT):
        ncols = min(IN_F - 128 * o, 128)
        cols = slice(128 * o, 128 * o + ncols)
        for t in range(2):
            pay = w1pay[127 * (o + t) : 127 * (o + t + 1), cols]  # [127, ncols]
            w1el[1:, o, t, :ncols] = pay
        w1el[0, o, 0, :ncols] = b1p[cols]
    w1el = np.ascontiguousarray(w1el.reshape(128, NT * 2 * 128)).astype(FP8)

    # w2: rows = permuted h1 features, zero-padded to 896; [p, kt, o] layout
    w2t = np.zeros((NT * 128, H2), dtype=F32)
    w2t[:IN_F] = ((np.asarray(W2, F32) * m2).T * SW)[P]
    w2l = np.ascontiguousarray(
        w2t.reshape(NT, 128, H2).transpose(1, 0, 2).reshape(128, NT * H2)
    ).astype(FP8)

    w3l = ((np.asarray(W3, F32) * m3).T).astype(BF16).copy()

    # bias pack [128, 1 + 10] f32: b2 scaled by SW^2; b3 broadcast.
    biasl = np.zeros((128, 1 + NCLS), F32)
    biasl[:, 0] = np.asarray(b2, F32) * (SW * SW)
    biasl[:, 1:] = np.asarray(b3, F32)[None, :]
    biasl = np.ascontiguousarray(biasl)

    # x: [B, 784] -> fp8, feature-permuted, payload+ones ext tiles,
    # batch permuted within each 2048-col block
    perm = _block_perm()
    full_perm = np.concatenate(
        [c * S + g * BLKC + perm for c in range(N_CORES) for g in range(NGRP)]
    )
    xT = np.asarray(x, F32).T.astype(FP8)[:, full_perm]
    xp = xT[P][ep]                       # [896, B] payload rows
    xe8 = np.zeros((NXT, 128, B), dtype=FP8)
    xe8[:, 0, :] = np.float32(1.0)
    pay = np.zeros((NXT * 127, B), dtype=FP8)
    pay[:NPAY] = xp
    xe8[:, 1:, :] = pay.reshape(NXT, 127, B)

    in_maps = []
    for c in range(N_CORES):
        xc = xe8[:, :, c * S : (c + 1) * S]
        # [NXT, 128, S] -> [NSB, 128, NXT, 512]: 4KB contiguous runs
        xc = np.ascontiguousarray(
            xc.reshape(NXT, 128, NSB, 512).transpose(2, 1, 0, 3)
        )
        in_maps.append(
            {
                "xe": xc,
                "w1e": w1el,
                "w2q": w2l,
                "w3q": w3l,
                "bias": biasl,
            }
        )
    return in_maps


def _run(inputs, trace=False, **run_kwargs):
    if "nc" not in _CACHE:
        nc = _build_nc()
        nc.finalize()
        _CACHE["nc"] = nc
    nc = _CACHE["nc"]
    in_maps = _prep_inputs(**inputs)
    res = run_bass_kernel_spmd(
        nc,
        in_maps,
        core_ids=list(range(N_CORES)),
        trace=trace,
        **run_kwargs,
    )
    out = np.concatenate([r["out"] for r in res.results], axis=0)
    return out, res


def kernel(**inputs):
    out, _ = _run(inputs, trace=False)
    return out
